# revision 47
# baseline (speedup 1.0000x reference)
"""Attention-pooling Trainium2 kernel (v5: M-trick + no V-proj + hybrid exp).

Problem: out = mean_s(softmax((x@Wq+bq)(x@Wk+bk)^T / sqrt(E)) @ (x@Wv+bv))
with x [4, 4096, 256], output [4, 1, 256].

Math restructuring (all exact up to fp reassociation):
  * scores = (x Wq + bq)(x Wk)^T = x M x^T + g . x_t with M = Wq Wk^T and
    g = bq Wk^T precomputed on the host: the device projects qm = x@M + g
    once and matmuls it straight against x^T -- the ENTIRE K projection
    disappears, and the Q bias folds exactly into the qm bias.
  * mean_s(dist @ V) = (colsum(dist)/S) @ V, and w @ (x@Wv) = (w @ x) @ Wv:
    the device only computes u = w @ x; the tiny [1,256]x[256,256] Wv map
    runs on the host -- the V projection also disappears.
  * K bias drops exactly (softmax row-shift invariance). V bias folds into
    the host epilogue.
  * No per-row max subtraction before exp: scores ~N(0,1), fp32 exp is safe.

Engine layout per core (PE ~73us is the critical path; ACT ~63; DVE ~55):
  * PE: qm projection (16 MMs), scores 256 MMs n=512 bf16 (stream-bound
    roofline ~215ns each), colsum via m=1 strip matmuls at 4 concurrent
    tile_position column groups accumulating w in 2 persistent PSUM banks,
    final u = w@x matvec.
  * ACT: exp of the two 1536-wide chunks per q-tile (PSUM 2-slot rotation:
    2x3 banks + 2 w banks = 8) with accum_out row-sums; qm bias-casts.
  * DVE: the 1024 chunk of every q-tile via the Schraudolph bit-trick
    exp(z) ~= bitcast_f32(int32(A*z + B)) (~3% per element, ~0.1% on the
    output); reciprocals and glue; one 32x32 block transpose of w feeding
    the tail (xn tiles are host-permuted to match).

Sharding: 8 cores = 4 batches x 2 query-row halves; core receives x[b].T
rolled so its own 2048 query rows are always columns 0:2047 (the key/value
permutation is harmless: w@x is permutation invariant since xn uses the
same roll). Host sums the two half partials, applies Wv, /S, +bv.
"""

import numpy as np

import concourse.bass as bass  # noqa: F401
import concourse.mybir as mybir
import concourse.tile as tile
from concourse import bacc

B, S, E = 4, 4096, 256
HALF = S // 2          # query rows per core
NORM = 16.0            # sqrt(E)
P = 128
N_CORES = 8
QTILES = HALF // P     # 16
F32 = mybir.dt.float32
BF16 = mybir.dt.bfloat16
I32 = mybir.dt.int32

XTW = 2048             # xt load chunk width (big DMAs; few per queue)
PROJW = 1024           # projection unit width

# scores chunks per q-tile: (t0, width, engine) -- "act" = LUT exp on ACT,
# "dve" = Schraudolph exp on DVE
CHUNKS = [(0, 1536, "act"), (1536, 1536, "act"), (3072, 1024, "dve")]
COLSUM_LAG = 2

# Schraudolph: exp(z) ~= bitcast_f32(int32(z * 2^23/ln2 + (127<<23) - C))
SCH_A = float(2.0**23 / np.log(2.0) / NORM)   # folds the 1/sqrt(E) scale
SCH_B = float(127 * 2**23 - 450000)


def _emit(ctx, tc):
    nc = tc.nc

    xt_d = nc.dram_tensor("xt", [E, S], BF16, kind="ExternalInput")
    # x in permuted [t, e] tiles for the final u = w @ x matvec (see
    # _in_maps for the permutation matching the DVE block transpose)
    xn_d = nc.dram_tensor("xn", [P, S // P, E], BF16, kind="ExternalInput")
    # M = Wq @ Wk^T, p-major so the DMA is contiguous: wall[p, c, e]
    # = M[128c + p, e]
    wall_d = nc.dram_tensor("wall", [P, 2, E], BF16, kind="ExternalInput")
    # g = bq @ Wk^T, pre-shaped [128, 2] so the DMA is contiguous
    bqc_d = nc.dram_tensor("bqc", [P, 2], F32, kind="ExternalInput")
    out_d = nc.dram_tensor("out", [4, E], F32, kind="ExternalOutput")

    const = ctx.enter_context(tc.tile_pool(name="const", bufs=1))
    epool = ctx.enter_context(tc.tile_pool(name="epool", bufs=4))
    rsp = ctx.enter_context(tc.tile_pool(name="rsp", bufs=3))
    ypool = ctx.enter_context(tc.tile_pool(name="ypool", bufs=2))
    pp = ctx.enter_context(tc.tile_pool(name="pp", bufs=2, space="PSUM"))
    wp = ctx.enter_context(tc.tile_pool(name="wp", bufs=1, space="PSUM"))

    # ---- small loads first so the projection can start immediately.
    wq_all = const.tile([P, 2, E], BF16, name="wq_all")
    # first chunks small: per-queue DMA bandwidth is only ~50GB/s, so the
    # first projection matmul gates on the first chunk's transfer time
    chunk_bounds = [(0, 512), (512, 512), (1024, 1024), (2048, 2048)]
    xt_sb = [[None] * len(chunk_bounds) for _ in range(2)]

    def xt_load(ci, ei, eng):
        t0, width = chunk_bounds[ci]
        t = const.tile([P, width], BF16, name=f"xt{ei}_{ci}", tag=f"xt{ei}_{ci}")
        eng.dma_start(out=t, in_=xt_d[ei * P : (ei + 1) * P, t0 : t0 + width])
        xt_sb[ei][ci] = t

    # two pieces: the first 128 columns of M unblock the first proj matmul
    # sooner (per-queue DMA transfer rate is only ~50GB/s)
    nc.sync.dma_start(out=wq_all[:, :, 0:P], in_=wall_d[:, :, 0:P])
    nc.sync.dma_start(out=wq_all[:, :, P:E], in_=wall_d[:, :, P:E])
    bqc_sb = const.tile([P, 2], F32, name="bqc_sb")
    nc.scalar.dma_start(out=bqc_sb, in_=bqc_d[:, :])
    # first ei0 chunk split across two queues so the first matmul's input
    # lands in ~half the transfer time; otherwise keep xt issues off scalar
    t00 = const.tile([P, 512], BF16, name="xt0_0", tag="xt0_0")
    nc.gpsimd.dma_start(out=t00[:, 0:256], in_=xt_d[0:P, 0:256])
    nc.scalar.dma_start(out=t00[:, 256:512], in_=xt_d[0:P, 256:512])
    xt_sb[0][0] = t00
    xt_load(0, 1, nc.sync)
    xt_load(1, 0, nc.scalar)
    xt_load(1, 1, nc.sync)
    for ci in range(2, len(chunk_bounds)):
        xt_load(ci, 0, nc.gpsimd if ci % 2 else nc.sync)
        xt_load(ci, 1, nc.sync if ci % 2 else nc.gpsimd)
    # preload the exp table set behind the critical DMA issues: the first
    # real exp would otherwise pay ~2.7us of ACT_TABLE_LOAD mid-loop
    dummy = const.tile([P, 1], F32, name="dummy")
    nc.vector.memset(dummy, 0.0)
    nc.scalar.activation(
        out=dummy, in_=dummy, func=mybir.ActivationFunctionType.Exp, scale=1.0
    )
    # xn (x in [t,e] tiles) is only needed by the tail matvec
    xn_sb = const.tile([P, S // P, E], BF16, name="xn_sb")
    for h in range(4):
        nc.gpsimd.dma_start(
            out=xn_sb[:, h * 8 : (h + 1) * 8, :],
            in_=xn_d[:, h * 8 : (h + 1) * 8, :],
        )

    # ---- qm = x @ M + g projection: bf16 matmuls, fp32 PSUM, ACT casts
    # back to bf16 (with the g bias) in [128, 2(e-chunk), s] layout.
    qt_sb = const.tile([P, 2, HALF], BF16, name="qt_sb")

    def xt_slice(ei, t0, width):
        for ci, (c0, cw) in enumerate(chunk_bounds):
            if c0 <= t0 and t0 + width <= c0 + cw:
                return xt_sb[ei][ci][:, t0 - c0 : t0 - c0 + width]
        raise AssertionError(f"xt slice [{t0}, {t0+width}) crosses chunk bounds")

    def proj_unit(eo, u):
        """qt_sb[:, eo, u*PROJW:(u+1)*PROJW] = (M.T chunk @ xt chunk) + g."""
        ps = pp.tile([P, 1536], F32, tag="ps", name=f"ps_qm{eo}_{u}")
        for ei in range(2):
            lhsT = wq_all[:, ei, eo * P : (eo + 1) * P]
            for h in range(PROJW // 512):
                nc.tensor.matmul(
                    ps[:, h * 512 : (h + 1) * 512],
                    lhsT,
                    xt_slice(ei, u * PROJW + h * 512, 512),
                    start=(ei == 0),
                    stop=(ei == 1),
                )
        # for u0, cast the first 128 cols separately so q-tile 0's scores
        # unblock ~1us sooner
        pieces = [(0, P), (P, PROJW - P)] if u == 0 else [(0, PROJW)]
        for o0, owid in pieces:
            nc.scalar.activation(
                out=qt_sb[:, eo, u * PROJW + o0 : u * PROJW + o0 + owid],
                in_=ps[:, o0 : o0 + owid],
                func=mybir.ActivationFunctionType.Identity,
                bias=bqc_sb[:, eo : eo + 1],
                scale=1.0,
            )

    # q-tiles 0..7 only need qm unit u0; u1 is deferred into the loop
    for eo in range(2):
        proj_unit(eo, 0)

    # ---- w accumulators: chunk m (512 wide) -> tile m//4, strip 32*(m%4).
    w_ps = [wp.tile([P, 512], F32, tag=f"w{i}", name=f"w_ps{i}") for i in range(2)]
    for i in range(2):
        nc.vector.memset(w_ps[i], 0.0)

    def w_slot(m):
        return w_ps[m // 4], 32 * (m % 4)

    def emit_chunk(qi, ci, Es, rs):
        t0, wid, eng = CHUNKS[ci]
        ps = pp.tile([P, 1536], F32, tag="ps", name=f"ps_s{qi}_{ci}")
        for h in range(wid // 512):
            c0 = t0 + h * 512
            for ei in range(2):
                nc.tensor.matmul(
                    ps[:, h * 512 : (h + 1) * 512],
                    qt_sb[:, ei, qi * P : (qi + 1) * P],
                    xt_slice(ei, c0, 512),
                    start=(ei == 0),
                    stop=(ei == 1),
                )
        if eng == "act":
            nc.scalar.activation(
                out=Es[:, t0 : t0 + wid],
                in_=ps[:, :wid],
                func=mybir.ActivationFunctionType.Exp,
                scale=1.0 / NORM,
                accum_out=rs[:, ci : ci + 1],
            )
        else:
            y = ypool.tile([P, 1024], I32, tag="y", name=f"y{qi}_{ci}")
            nc.vector.tensor_scalar(
                y[:, :wid],
                ps[:, :wid],
                SCH_A,
                SCH_B,
                mybir.AluOpType.mult,
                mybir.AluOpType.add,
            )
            nc.vector.tensor_scalar(
                Es[:, t0 : t0 + wid],
                y.bitcast(F32)[:, :wid],
                1.0,
                None,
                mybir.AluOpType.mult,
                mybir.AluOpType.add,
                accum_out=rs[:, ci : ci + 1],
            )

    def emit_recip(qi, rs):
        rsum = rsp.tile([P, 1], F32, tag="rsum", name=f"rsum{qi}")
        nc.vector.reduce_sum(out=rsum, in_=rs, axis=mybir.AxisListType.X)
        recf = rsp.tile([P, 1], F32, tag="recf", name=f"recf{qi}")
        nc.vector.reciprocal(out=recf, in_=rsum)
        recb = rsp.tile([P, 1], BF16, tag="recb", name=f"recb{qi}")
        nc.vector.tensor_copy(out=recb, in_=recf)
        return recb

    def emit_colsum(qi, Es, recb, ms=range(8)):
        for m in ms:
            wt, strip = w_slot(m)
            nc.tensor.matmul(
                wt[strip : strip + 1, :],
                recb,
                Es[:, m * 512 : (m + 1) * 512],
                start=(qi == 0),
                stop=(qi == QTILES - 1),
                tile_position=(0, strip),
            )

    # software-pipeline colsum: its recip dependency chain (accum reads ->
    # reduce -> reciprocal -> cast) takes a few us; lag a couple of q-tiles
    pending = {}
    for qi in range(QTILES):
        Es = epool.tile([P, S], BF16, tag="E", name=f"E{qi}")
        rs = rsp.tile([P, len(CHUNKS)], F32, tag="rs", name=f"rs{qi}")
        # order c0 (ACT), c2 (DVE), colsum, c1 (ACT): the two slots' next
        # writers arrive after their consumers' release, and c0 first keeps
        # q-tile 0 off the critical path of the late xt DMA chunks
        emit_chunk(qi, 0, Es, rs)
        emit_chunk(qi, 2, Es, rs)
        if qi - COLSUM_LAG in pending:
            emit_colsum(qi - COLSUM_LAG, *pending.pop(qi - COLSUM_LAG))
        if qi == QTILES - 1 and qi - 1 in pending:
            # drain the backlog so only q-tile 15's colsum trails the loop
            emit_colsum(qi - 1, *pending.pop(qi - 1))
        emit_chunk(qi, 1, Es, rs)
        if qi == 1:
            # fills the PE during the early pipeline-warmup stalls
            for eo in range(2):
                proj_unit(eo, 1)
        pending[qi] = (Es, emit_recip(qi, rs))
    for qi in sorted(pending):
        emit_colsum(qi, *pending.pop(qi))

    # ---- tail: w strips -> SBUF, DVE 32x32 block-transpose to w^T columns,
    # matvec against xn (see _in_maps for the index mapping). Processed in
    # two halves (w_ps[0] -> matvec cols 0:16, w_ps[1] -> 16:32) so the
    # first half's matvec overlaps the second half's transpose.
    w_sb_f = const.tile([P, 1024], F32, name="w_sb_f")
    w_tr = const.tile([P, 1024], F32, name="w_tr")
    wt_sb = const.tile([P, 32], BF16, name="wt_sb")
    fin = None
    for half in range(2):
        h0 = half * 512
        if half == 1:
            # ACT copies the second half in parallel with DVE's first-half
            # transpose chain
            nc.scalar.copy(out=w_sb_f[:, h0 : h0 + 512], in_=w_ps[half])
        else:
            nc.vector.tensor_copy(out=w_sb_f[:, h0 : h0 + 512], in_=w_ps[half])
        if half == 0:
            # reuses w_ps[0]'s bank -- safe: it has just been copied out
            fin = wp.tile([P, 512], F32, tag="w0", name="fin")
            nc.vector.memset(fin[:, 0:E], 0.0)
        nc.vector.transpose(
            out=w_tr[:, h0 : h0 + 512], in_=w_sb_f[:, h0 : h0 + 512]
        )
        src = bass.AP(
            tensor=w_tr.tensor,
            offset=w_tr.offset + h0,
            ap=[w_tr.ap[0], [32, 16]],
        )
        nc.vector.tensor_copy(out=wt_sb[:, half * 16 : half * 16 + 16], in_=src)
        # final matvec u = w @ x: 4 concurrent column strips accumulate
        # partials at partitions {0,32,64,96}; combined on the host
        for c in range(16):
            col = half * 16 + c
            strip = 32 * (col % 4)
            nc.tensor.matmul(
                fin[strip : strip + 1, 0:E],
                wt_sb[:, col : col + 1],
                xn_sb[:, col, :],
                start=(col < 4),
                stop=(col >= 28),
                tile_position=(0, strip),
            )
    strips_sb = const.tile([P, E], F32, name="strips_sb")
    nc.vector.tensor_copy(out=strips_sb, in_=fin[:, 0:E])
    src_ap = bass.AP(
        tensor=strips_sb.tensor,
        offset=strips_sb.offset,
        ap=[[strips_sb.ap[0][0] * 32, 4], [1, E]],
    )
    nc.sync.dma_start(out=out_d[:, :], in_=src_ap)


_NC_CACHE = None


def _build_nc():
    global _NC_CACHE
    if _NC_CACHE is None:
        from contextlib import ExitStack

        nc = bacc.Bacc("TRN2", target_bir_lowering=False, debug=False)
        with tile.TileContext(nc) as tc, ExitStack() as ctx:
            _emit(ctx, tc)
        nc.compile()
        _NC_CACHE = nc
    return _NC_CACHE


def _in_maps(inputs):
    import ml_dtypes

    bf16 = ml_dtypes.bfloat16
    x = np.asarray(inputs["x"], dtype=np.float32)
    Wq = np.asarray(inputs["Wq"], dtype=np.float32)
    Wk = np.asarray(inputs["Wk"], dtype=np.float32)
    M = (Wq @ Wk.T).astype(np.float32)
    g = (np.asarray(inputs["bq"], dtype=np.float32) @ Wk.T).astype(np.float32)
    # p-major: wall[p, c, e] = M[128c + p, e]
    wall = np.ascontiguousarray(
        np.stack([M[:P, :], M[P:, :]], axis=1)
    ).astype(bf16)
    # bqc[p, a] = g[128a + p]
    bqc = np.ascontiguousarray(g.reshape(2, P).T)
    # xn tile cc, partition p holds x row t = 2048(cc//16) + 512(p//32)
    # + 32(cc%16) + (p%32), matching the DVE-block-transposed w layout
    p = np.arange(P)[:, None]
    cc = np.arange(S // P)[None, :]
    tidx = 2048 * (cc // 16) + 512 * (p // 32) + 32 * (cc % 16) + (p % 32)
    maps = []
    for c in range(N_CORES):
        b, h = divmod(c, 2)
        xr = np.roll(x[b], -h * HALF, axis=0)
        xt = np.ascontiguousarray(xr.T).astype(bf16)
        xn = np.ascontiguousarray(xr[tidx]).astype(bf16)
        maps.append({"xt": xt, "xn": xn, "wall": wall, "bqc": bqc})
    return maps


def _combine(results, inputs):
    Wv = np.asarray(inputs["Wv"], dtype=np.float32)
    bv = np.asarray(inputs["bv"], dtype=np.float32).reshape(E)
    parts = [r["out"].sum(axis=0) for r in results]
    out = np.stack(
        [((parts[2 * b] + parts[2 * b + 1]) / S) @ Wv + bv for b in range(B)]
    )[:, None, :]
    return out.astype(np.float32)


def kernel(**inputs):
    from concourse.bass_utils import run_bass_kernel_spmd

    nc = _build_nc()
    res = run_bass_kernel_spmd(nc, _in_maps(inputs), core_ids=list(range(N_CORES)))
    return _combine(res.results, inputs)


# revision 49
# speedup vs baseline: 1.0318x; 1.0318x over previous
"""Attention-pooling Trainium2 kernel (v5: M-trick + no V-proj + hybrid exp).

Problem: out = mean_s(softmax((x@Wq+bq)(x@Wk+bk)^T / sqrt(E)) @ (x@Wv+bv))
with x [4, 4096, 256], output [4, 1, 256].

Math restructuring (all exact up to fp reassociation):
  * scores = (x Wq + bq)(x Wk)^T = x M x^T + g . x_t with M = Wq Wk^T and
    g = bq Wk^T precomputed on the host: the device projects qm = x@M + g
    once and matmuls it straight against x^T -- the ENTIRE K projection
    disappears, and the Q bias folds exactly into the qm bias.
  * mean_s(dist @ V) = (colsum(dist)/S) @ V, and w @ (x@Wv) = (w @ x) @ Wv:
    the device only computes u = w @ x; the tiny [1,256]x[256,256] Wv map
    runs on the host -- the V projection also disappears.
  * K bias drops exactly (softmax row-shift invariance). V bias folds into
    the host epilogue.
  * No per-row max subtraction before exp: scores ~N(0,1), fp32 exp is safe.

Engine layout per core (PE ~73us is the critical path; ACT ~63; DVE ~55):
  * PE: qm projection (16 MMs), scores 256 MMs n=512 bf16 (stream-bound
    roofline ~215ns each), colsum via m=1 strip matmuls at 4 concurrent
    tile_position column groups accumulating w in 2 persistent PSUM banks,
    final u = w@x matvec.
  * ACT: exp of the two 1536-wide chunks per q-tile (PSUM 2-slot rotation:
    2x3 banks + 2 w banks = 8) with accum_out row-sums; qm bias-casts.
  * DVE: the 1024 chunk of every q-tile via the Schraudolph bit-trick
    exp(z) ~= bitcast_f32(int32(A*z + B)) (~3% per element, ~0.1% on the
    output); reciprocals and glue; one 32x32 block transpose of w feeding
    the tail (xn tiles are host-permuted to match).

Sharding: 8 cores = 4 batches x 2 query-row halves; core receives x[b].T
rolled so its own 2048 query rows are always columns 0:2047 (the key/value
permutation is harmless: w@x is permutation invariant since xn uses the
same roll). Host sums the two half partials, applies Wv, /S, +bv.
"""

import numpy as np

import concourse.bass as bass  # noqa: F401
import concourse.mybir as mybir
import concourse.tile as tile
from concourse import bacc

B, S, E = 4, 4096, 256
HALF = S // 2          # query rows per core
NORM = 16.0            # sqrt(E)
P = 128
N_CORES = 8
QTILES = HALF // P     # 16
F32 = mybir.dt.float32
BF16 = mybir.dt.bfloat16
I32 = mybir.dt.int32

XTW = 2048             # xt load chunk width (big DMAs; few per queue)
PROJW = 1024           # projection unit width

# scores chunks per q-tile: (t0, width, engine) -- "act" = LUT exp on ACT,
# "dve" = Schraudolph exp on DVE
CHUNKS = [(0, 1536, "act"), (1536, 1536, "act"), (3072, 1024, "dve")]
COLSUM_LAG = 2

# Schraudolph: exp(z) ~= bitcast_f32(int32(z * 2^23/ln2 + (127<<23) - C))
SCH_A = float(2.0**23 / np.log(2.0) / NORM)   # folds the 1/sqrt(E) scale
SCH_B = float(127 * 2**23 - 450000)


def _emit(ctx, tc):
    nc = tc.nc

    xt_d = nc.dram_tensor("xt", [E, S], BF16, kind="ExternalInput")
    # x in permuted [t, e] tiles for the final u = w @ x matvec (see
    # _in_maps for the permutation matching the DVE block transpose)
    xn_d = nc.dram_tensor("xn", [P, S // P, E], BF16, kind="ExternalInput")
    # M = Wq @ Wk^T, p-major so the DMA is contiguous: wall[p, c, e]
    # = M[128c + p, e]
    wall_d = nc.dram_tensor("wall", [P, 2, E], BF16, kind="ExternalInput")
    # g = bq @ Wk^T, pre-shaped [128, 2] so the DMA is contiguous
    bqc_d = nc.dram_tensor("bqc", [P, 2], F32, kind="ExternalInput")
    out_d = nc.dram_tensor("out", [4, E], F32, kind="ExternalOutput")

    const = ctx.enter_context(tc.tile_pool(name="const", bufs=1))
    epool = ctx.enter_context(tc.tile_pool(name="epool", bufs=4))
    rsp = ctx.enter_context(tc.tile_pool(name="rsp", bufs=3))
    ypool = ctx.enter_context(tc.tile_pool(name="ypool", bufs=2))
    pp = ctx.enter_context(tc.tile_pool(name="pp", bufs=2, space="PSUM"))
    wp = ctx.enter_context(tc.tile_pool(name="wp", bufs=1, space="PSUM"))

    # ---- small loads first so the projection can start immediately.
    wq_all = const.tile([P, 2, E], BF16, name="wq_all")
    # first chunks small: per-queue DMA bandwidth is only ~50GB/s, so the
    # first projection matmul gates on the first chunk's transfer time
    chunk_bounds = [(0, 512), (512, 512), (1024, 1024), (2048, 2048)]
    xt_sb = [[None] * len(chunk_bounds) for _ in range(2)]

    def xt_load(ci, ei, eng):
        t0, width = chunk_bounds[ci]
        t = const.tile([P, width], BF16, name=f"xt{ei}_{ci}", tag=f"xt{ei}_{ci}")
        eng.dma_start(out=t, in_=xt_d[ei * P : (ei + 1) * P, t0 : t0 + width])
        xt_sb[ei][ci] = t

    # keep ALL xt transfers OFF the scalar queue: scalar must reach the
    # exp-table preload and qt casts quickly (measured: polluting scalar
    # with xt DMAs costs ~4-5us at warm clock)
    nc.sync.dma_start(out=wq_all, in_=wall_d[:, :, :])
    bqc_sb = const.tile([P, 2], F32, name="bqc_sb")
    nc.scalar.dma_start(out=bqc_sb, in_=bqc_d[:, :])
    xt_load(0, 0, nc.gpsimd)
    xt_load(0, 1, nc.sync)
    xt_load(1, 0, nc.gpsimd)
    xt_load(1, 1, nc.sync)
    for ci in range(2, len(chunk_bounds)):
        xt_load(ci, 0, nc.gpsimd if ci % 2 else nc.sync)
        xt_load(ci, 1, nc.sync if ci % 2 else nc.gpsimd)
    # preload the exp table set behind the critical DMA issues: the first
    # real exp would otherwise pay ~2.7us of ACT_TABLE_LOAD mid-loop
    dummy = const.tile([P, 1], F32, name="dummy")
    nc.vector.memset(dummy, 0.0)
    nc.scalar.activation(
        out=dummy, in_=dummy, func=mybir.ActivationFunctionType.Exp, scale=1.0
    )
    # xn (x in [t,e] tiles) is only needed by the tail matvec
    xn_sb = const.tile([P, S // P, E], BF16, name="xn_sb")
    for h in range(4):
        nc.gpsimd.dma_start(
            out=xn_sb[:, h * 8 : (h + 1) * 8, :],
            in_=xn_d[:, h * 8 : (h + 1) * 8, :],
        )

    # ---- qm = x @ M + g projection: bf16 matmuls, fp32 PSUM, ACT casts
    # back to bf16 (with the g bias) in [128, 2(e-chunk), s] layout.
    qt_sb = const.tile([P, 2, HALF], BF16, name="qt_sb")

    def xt_slice(ei, t0, width):
        for ci, (c0, cw) in enumerate(chunk_bounds):
            if c0 <= t0 and t0 + width <= c0 + cw:
                return xt_sb[ei][ci][:, t0 - c0 : t0 - c0 + width]
        raise AssertionError(f"xt slice [{t0}, {t0+width}) crosses chunk bounds")

    def proj_unit(eo, u):
        """qt_sb[:, eo, u*PROJW:(u+1)*PROJW] = (M.T chunk @ xt chunk) + g."""
        ps = pp.tile([P, 1536], F32, tag="ps", name=f"ps_qm{eo}_{u}")
        for ei in range(2):
            lhsT = wq_all[:, ei, eo * P : (eo + 1) * P]
            for h in range(PROJW // 512):
                nc.tensor.matmul(
                    ps[:, h * 512 : (h + 1) * 512],
                    lhsT,
                    xt_slice(ei, u * PROJW + h * 512, 512),
                    start=(ei == 0),
                    stop=(ei == 1),
                )
        if u == 0:
            # cast the first 128 cols separately so q-tile 0's scores
            # unblock ~1us sooner; ACT is idle during the ramp
            for o0, owid in ((0, P), (P, PROJW - P)):
                nc.scalar.activation(
                    out=qt_sb[:, eo, o0 : o0 + owid],
                    in_=ps[:, o0 : o0 + owid],
                    func=mybir.ActivationFunctionType.Identity,
                    bias=bqc_sb[:, eo : eo + 1],
                    scale=1.0,
                )
        else:
            # u1 is emitted mid-loop (qi==1): cast on DVE so ACT's exp
            # stream (which gates PE slot reuse) is not interrupted
            nc.vector.tensor_scalar(
                qt_sb[:, eo, PROJW : 2 * PROJW],
                ps[:, :PROJW],
                bqc_sb[:, eo : eo + 1],
                None,
                mybir.AluOpType.add,
            )

    # q-tiles 0..7 only need qm unit u0; u1 is deferred into the loop
    for eo in range(2):
        proj_unit(eo, 0)

    # ---- w accumulators: chunk m (512 wide) -> tile m//4, strip 32*(m%4).
    w_ps = [wp.tile([P, 512], F32, tag=f"w{i}", name=f"w_ps{i}") for i in range(2)]
    for i in range(2):
        nc.vector.memset(w_ps[i], 0.0)

    def w_slot(m):
        return w_ps[m // 4], 32 * (m % 4)

    def emit_chunk(qi, ci, Es, rs):
        t0, wid, eng = CHUNKS[ci]
        ps = pp.tile([P, 1536], F32, tag="ps", name=f"ps_s{qi}_{ci}")
        for h in range(wid // 512):
            c0 = t0 + h * 512
            for ei in range(2):
                nc.tensor.matmul(
                    ps[:, h * 512 : (h + 1) * 512],
                    qt_sb[:, ei, qi * P : (qi + 1) * P],
                    xt_slice(ei, c0, 512),
                    start=(ei == 0),
                    stop=(ei == 1),
                )
        if eng == "act":
            nc.scalar.activation(
                out=Es[:, t0 : t0 + wid],
                in_=ps[:, :wid],
                func=mybir.ActivationFunctionType.Exp,
                scale=1.0 / NORM,
                accum_out=rs[:, ci : ci + 1],
            )
        else:
            y = ypool.tile([P, 1024], I32, tag="y", name=f"y{qi}_{ci}")
            nc.vector.tensor_scalar(
                y[:, :wid],
                ps[:, :wid],
                SCH_A,
                SCH_B,
                mybir.AluOpType.mult,
                mybir.AluOpType.add,
            )
            nc.vector.tensor_scalar(
                Es[:, t0 : t0 + wid],
                y.bitcast(F32)[:, :wid],
                1.0,
                None,
                mybir.AluOpType.mult,
                mybir.AluOpType.add,
                accum_out=rs[:, ci : ci + 1],
            )

    def emit_recip(qi, rs):
        rsum = rsp.tile([P, 1], F32, tag="rsum", name=f"rsum{qi}")
        nc.vector.reduce_sum(out=rsum, in_=rs, axis=mybir.AxisListType.X)
        recf = rsp.tile([P, 1], F32, tag="recf", name=f"recf{qi}")
        nc.vector.reciprocal(out=recf, in_=rsum)
        recb = rsp.tile([P, 1], BF16, tag="recb", name=f"recb{qi}")
        nc.vector.tensor_copy(out=recb, in_=recf)
        return recb

    def emit_colsum(qi, Es, recb, ms=range(8)):
        for m in ms:
            wt, strip = w_slot(m)
            nc.tensor.matmul(
                wt[strip : strip + 1, :],
                recb,
                Es[:, m * 512 : (m + 1) * 512],
                start=(qi == 0),
                stop=(qi == QTILES - 1),
                tile_position=(0, strip),
            )

    # software-pipeline colsum: its recip dependency chain (accum reads ->
    # reduce -> reciprocal -> cast) takes a few us; lag a couple of q-tiles
    pending = {}
    for qi in range(QTILES):
        Es = epool.tile([P, S], BF16, tag="E", name=f"E{qi}")
        rs = rsp.tile([P, len(CHUNKS)], F32, tag="rs", name=f"rs{qi}")
        # order c0 (ACT), c2 (DVE), colsum, c1 (ACT): the two slots' next
        # writers arrive after their consumers' release, and c0 first keeps
        # q-tile 0 off the critical path of the late xt DMA chunks
        emit_chunk(qi, 0, Es, rs)
        emit_chunk(qi, 2, Es, rs)
        if qi - COLSUM_LAG in pending:
            emit_colsum(qi - COLSUM_LAG, *pending.pop(qi - COLSUM_LAG))
        if qi == QTILES - 1 and qi - 1 in pending:
            # drain the backlog so only q-tile 15's colsum trails the loop
            emit_colsum(qi - 1, *pending.pop(qi - 1))
        emit_chunk(qi, 1, Es, rs)
        if qi == 1:
            # fills the PE during the early pipeline-warmup stalls
            for eo in range(2):
                proj_unit(eo, 1)
        pending[qi] = (Es, emit_recip(qi, rs))
    for qi in sorted(pending):
        emit_colsum(qi, *pending.pop(qi))

    # ---- tail: w strips -> SBUF, DVE 32x32 block-transpose to w^T columns,
    # matvec against xn (see _in_maps for the index mapping). Processed in
    # two halves (w_ps[0] -> matvec cols 0:16, w_ps[1] -> 16:32) so the
    # first half's matvec overlaps the second half's transpose.
    w_sb_f = const.tile([P, 1024], F32, name="w_sb_f")
    w_tr = const.tile([P, 1024], F32, name="w_tr")
    wt_sb = const.tile([P, 32], BF16, name="wt_sb")
    fin = None
    for half in range(2):
        h0 = half * 512
        if half == 1:
            # ACT copies the second half in parallel with DVE's first-half
            # transpose chain
            nc.scalar.copy(out=w_sb_f[:, h0 : h0 + 512], in_=w_ps[half])
        else:
            nc.vector.tensor_copy(out=w_sb_f[:, h0 : h0 + 512], in_=w_ps[half])
        if half == 0:
            # reuses w_ps[0]'s bank -- safe: it has just been copied out
            fin = wp.tile([P, 512], F32, tag="w0", name="fin")
            nc.vector.memset(fin[:, 0:E], 0.0)
        nc.vector.transpose(
            out=w_tr[:, h0 : h0 + 512], in_=w_sb_f[:, h0 : h0 + 512]
        )
        src = bass.AP(
            tensor=w_tr.tensor,
            offset=w_tr.offset + h0,
            ap=[w_tr.ap[0], [32, 16]],
        )
        nc.vector.tensor_copy(out=wt_sb[:, half * 16 : half * 16 + 16], in_=src)
        # final matvec u = w @ x: 4 concurrent column strips accumulate
        # partials at partitions {0,32,64,96}; combined on the host
        for c in range(16):
            col = half * 16 + c
            strip = 32 * (col % 4)
            nc.tensor.matmul(
                fin[strip : strip + 1, 0:E],
                wt_sb[:, col : col + 1],
                xn_sb[:, col, :],
                start=(col < 4),
                stop=(col >= 28),
                tile_position=(0, strip),
            )
    strips_sb = const.tile([P, E], F32, name="strips_sb")
    nc.vector.tensor_copy(out=strips_sb, in_=fin[:, 0:E])
    src_ap = bass.AP(
        tensor=strips_sb.tensor,
        offset=strips_sb.offset,
        ap=[[strips_sb.ap[0][0] * 32, 4], [1, E]],
    )
    nc.sync.dma_start(out=out_d[:, :], in_=src_ap)


_NC_CACHE = None


def _build_nc():
    global _NC_CACHE
    if _NC_CACHE is None:
        from contextlib import ExitStack

        nc = bacc.Bacc("TRN2", target_bir_lowering=False, debug=False)
        with tile.TileContext(nc) as tc, ExitStack() as ctx:
            _emit(ctx, tc)
        nc.compile()
        _NC_CACHE = nc
    return _NC_CACHE


def _in_maps(inputs):
    import ml_dtypes

    bf16 = ml_dtypes.bfloat16
    x = np.asarray(inputs["x"], dtype=np.float32)
    Wq = np.asarray(inputs["Wq"], dtype=np.float32)
    Wk = np.asarray(inputs["Wk"], dtype=np.float32)
    M = (Wq @ Wk.T).astype(np.float32)
    g = (np.asarray(inputs["bq"], dtype=np.float32) @ Wk.T).astype(np.float32)
    # p-major: wall[p, c, e] = M[128c + p, e]
    wall = np.ascontiguousarray(
        np.stack([M[:P, :], M[P:, :]], axis=1)
    ).astype(bf16)
    # bqc[p, a] = g[128a + p]
    bqc = np.ascontiguousarray(g.reshape(2, P).T)
    # xn tile cc, partition p holds x row t = 2048(cc//16) + 512(p//32)
    # + 32(cc%16) + (p%32), matching the DVE-block-transposed w layout
    p = np.arange(P)[:, None]
    cc = np.arange(S // P)[None, :]
    tidx = 2048 * (cc // 16) + 512 * (p // 32) + 32 * (cc % 16) + (p % 32)
    maps = []
    for c in range(N_CORES):
        b, h = divmod(c, 2)
        xr = np.roll(x[b], -h * HALF, axis=0)
        xt = np.ascontiguousarray(xr.T).astype(bf16)
        xn = np.ascontiguousarray(xr[tidx]).astype(bf16)
        maps.append({"xt": xt, "xn": xn, "wall": wall, "bqc": bqc})
    return maps


def _combine(results, inputs):
    Wv = np.asarray(inputs["Wv"], dtype=np.float32)
    bv = np.asarray(inputs["bv"], dtype=np.float32).reshape(E)
    parts = [r["out"].sum(axis=0) for r in results]
    out = np.stack(
        [((parts[2 * b] + parts[2 * b + 1]) / S) @ Wv + bv for b in range(B)]
    )[:, None, :]
    return out.astype(np.float32)


def kernel(**inputs):
    from concourse.bass_utils import run_bass_kernel_spmd

    nc = _build_nc()
    res = run_bass_kernel_spmd(nc, _in_maps(inputs), core_ids=list(range(N_CORES)))
    return _combine(res.results, inputs)


# revision 50
# speedup vs baseline: 1.0464x; 1.0142x over previous
"""Attention-pooling Trainium2 kernel (v5: M-trick + no V-proj + hybrid exp).

Problem: out = mean_s(softmax((x@Wq+bq)(x@Wk+bk)^T / sqrt(E)) @ (x@Wv+bv))
with x [4, 4096, 256], output [4, 1, 256].

Math restructuring (all exact up to fp reassociation):
  * scores = (x Wq + bq)(x Wk)^T = x M x^T + g . x_t with M = Wq Wk^T and
    g = bq Wk^T precomputed on the host: the device projects qm = x@M + g
    once and matmuls it straight against x^T -- the ENTIRE K projection
    disappears, and the Q bias folds exactly into the qm bias.
  * mean_s(dist @ V) = (colsum(dist)/S) @ V, and w @ (x@Wv) = (w @ x) @ Wv:
    the device only computes u = w @ x; the tiny [1,256]x[256,256] Wv map
    runs on the host -- the V projection also disappears.
  * K bias drops exactly (softmax row-shift invariance). V bias folds into
    the host epilogue.
  * No per-row max subtraction before exp: scores ~N(0,1), fp32 exp is safe.

Engine layout per core (PE ~73us is the critical path; ACT ~63; DVE ~55):
  * PE: qm projection (16 MMs), scores 256 MMs n=512 bf16 (stream-bound
    roofline ~215ns each), colsum via m=1 strip matmuls at 4 concurrent
    tile_position column groups accumulating w in 2 persistent PSUM banks,
    final u = w@x matvec.
  * ACT: exp of the two 1536-wide chunks per q-tile (PSUM 2-slot rotation:
    2x3 banks + 2 w banks = 8) with accum_out row-sums; qm bias-casts.
  * DVE: the 1024 chunk of every q-tile via the Schraudolph bit-trick
    exp(z) ~= bitcast_f32(int32(A*z + B)) (~3% per element, ~0.1% on the
    output); reciprocals and glue; one 32x32 block transpose of w feeding
    the tail (xn tiles are host-permuted to match).

Sharding: 8 cores = 4 batches x 2 query-row halves; core receives x[b].T
rolled so its own 2048 query rows are always columns 0:2047 (the key/value
permutation is harmless: w@x is permutation invariant since xn uses the
same roll). Host sums the two half partials, applies Wv, /S, +bv.
"""

import numpy as np

import concourse.bass as bass  # noqa: F401
import concourse.mybir as mybir
import concourse.tile as tile
from concourse import bacc

B, S, E = 4, 4096, 256
HALF = S // 2          # query rows per core
NORM = 16.0            # sqrt(E)
P = 128
N_CORES = 8
QTILES = HALF // P     # 16
F32 = mybir.dt.float32
BF16 = mybir.dt.bfloat16
I32 = mybir.dt.int32

XTW = 2048             # xt load chunk width (big DMAs; few per queue)
PROJW = 1024           # projection unit width

# scores chunks per q-tile: (t0, width, engine) -- "act" = LUT exp on ACT,
# "dve" = Schraudolph exp on DVE
CHUNKS = [(0, 1536, "act"), (1536, 1536, "act"), (3072, 1024, "dve")]
COLSUM_LAG = 2

# Schraudolph: exp(z) ~= bitcast_f32(int32(z * 2^23/ln2 + (127<<23) - C))
SCH_A = float(2.0**23 / np.log(2.0) / NORM)   # folds the 1/sqrt(E) scale
SCH_B = float(127 * 2**23 - 450000)


def _emit(ctx, tc):
    nc = tc.nc

    xt_d = nc.dram_tensor("xt", [E, S], BF16, kind="ExternalInput")
    # x in permuted [t, e] tiles for the final u = w @ x matvec (see
    # _in_maps for the permutation matching the DVE block transpose)
    xn_d = nc.dram_tensor("xn", [P, S // P, E], BF16, kind="ExternalInput")
    # M = Wq @ Wk^T, p-major so the DMA is contiguous: wall[p, c, e]
    # = M[128c + p, e]
    wall_d = nc.dram_tensor("wall", [P, 2, E], BF16, kind="ExternalInput")
    # g = bq @ Wk^T, pre-shaped [128, 2] so the DMA is contiguous
    bqc_d = nc.dram_tensor("bqc", [P, 2], F32, kind="ExternalInput")
    out_d = nc.dram_tensor("out", [4, E], F32, kind="ExternalOutput")

    const = ctx.enter_context(tc.tile_pool(name="const", bufs=1))
    epool = ctx.enter_context(tc.tile_pool(name="epool", bufs=4))
    rsp = ctx.enter_context(tc.tile_pool(name="rsp", bufs=3))
    ypool = ctx.enter_context(tc.tile_pool(name="ypool", bufs=2))
    pp = ctx.enter_context(tc.tile_pool(name="pp", bufs=2, space="PSUM"))
    wp = ctx.enter_context(tc.tile_pool(name="wp", bufs=1, space="PSUM"))

    # ---- small loads first so the projection can start immediately.
    wq_all = const.tile([P, 2, E], BF16, name="wq_all")
    # first chunks small: per-queue DMA bandwidth is only ~50GB/s, so the
    # first projection matmul gates on the first chunk's transfer time
    chunk_bounds = [(0, 512), (512, 512), (1024, 1024), (2048, 2048)]
    xt_sb = [[None] * len(chunk_bounds) for _ in range(2)]

    def xt_load(ci, ei, eng):
        t0, width = chunk_bounds[ci]
        t = const.tile([P, width], BF16, name=f"xt{ei}_{ci}", tag=f"xt{ei}_{ci}")
        eng.dma_start(out=t, in_=xt_d[ei * P : (ei + 1) * P, t0 : t0 + width])
        xt_sb[ei][ci] = t

    # keep ALL xt transfers OFF the scalar queue: scalar must reach the
    # exp-table preload and qt casts quickly (measured: polluting scalar
    # with xt DMAs costs ~4-5us at warm clock)
    nc.sync.dma_start(out=wq_all, in_=wall_d[:, :, :])
    bqc_sb = const.tile([P, 2], F32, name="bqc_sb")
    nc.scalar.dma_start(out=bqc_sb, in_=bqc_d[:, :])
    xt_load(0, 0, nc.gpsimd)
    xt_load(0, 1, nc.sync)
    xt_load(1, 0, nc.gpsimd)
    xt_load(1, 1, nc.sync)
    for ci in range(2, len(chunk_bounds)):
        xt_load(ci, 0, nc.gpsimd if ci % 2 else nc.sync)
        xt_load(ci, 1, nc.sync if ci % 2 else nc.gpsimd)
    # preload the exp table set behind the critical DMA issues: the first
    # real exp would otherwise pay ~2.7us of ACT_TABLE_LOAD mid-loop
    dummy = const.tile([P, 1], F32, name="dummy")
    nc.vector.memset(dummy, 0.0)
    nc.scalar.activation(
        out=dummy, in_=dummy, func=mybir.ActivationFunctionType.Exp, scale=1.0
    )
    # xn (x in [t,e] tiles) is only needed by the tail matvec
    xn_sb = const.tile([P, S // P, E], BF16, name="xn_sb")
    for h in range(4):
        nc.gpsimd.dma_start(
            out=xn_sb[:, h * 8 : (h + 1) * 8, :],
            in_=xn_d[:, h * 8 : (h + 1) * 8, :],
        )

    # ---- qm = x @ M + g projection: bf16 matmuls, fp32 PSUM, ACT casts
    # back to bf16 (with the g bias) in [128, 2(e-chunk), s] layout.
    qt_sb = const.tile([P, 2, HALF], BF16, name="qt_sb")

    def xt_slice(ei, t0, width):
        for ci, (c0, cw) in enumerate(chunk_bounds):
            if c0 <= t0 and t0 + width <= c0 + cw:
                return xt_sb[ei][ci][:, t0 - c0 : t0 - c0 + width]
        raise AssertionError(f"xt slice [{t0}, {t0+width}) crosses chunk bounds")

    def proj_unit(eo, u):
        """qt_sb[:, eo, u*PROJW:(u+1)*PROJW] = (M.T chunk @ xt chunk) + g."""
        ps = pp.tile([P, 1536], F32, tag="ps", name=f"ps_qm{eo}_{u}")
        for ei in range(2):
            lhsT = wq_all[:, ei, eo * P : (eo + 1) * P]
            for h in range(PROJW // 512):
                nc.tensor.matmul(
                    ps[:, h * 512 : (h + 1) * 512],
                    lhsT,
                    xt_slice(ei, u * PROJW + h * 512, 512),
                    start=(ei == 0),
                    stop=(ei == 1),
                )
        if u == 0:
            # cast the first 128 cols separately so q-tile 0's scores
            # unblock ~1us sooner; ACT is idle during the ramp
            for o0, owid in ((0, P), (P, PROJW - P)):
                nc.scalar.activation(
                    out=qt_sb[:, eo, o0 : o0 + owid],
                    in_=ps[:, o0 : o0 + owid],
                    func=mybir.ActivationFunctionType.Identity,
                    bias=bqc_sb[:, eo : eo + 1],
                    scale=1.0,
                )
        else:
            # u1 is emitted mid-loop (qi==1): cast on DVE so ACT's exp
            # stream (which gates PE slot reuse) is not interrupted
            nc.vector.tensor_scalar(
                qt_sb[:, eo, PROJW : 2 * PROJW],
                ps[:, :PROJW],
                bqc_sb[:, eo : eo + 1],
                None,
                mybir.AluOpType.add,
            )

    # q-tiles 0..7 only need qm unit u0; u1 is deferred into the loop
    for eo in range(2):
        proj_unit(eo, 0)

    # ---- w accumulators: chunk m (512 wide) -> tile m//4, strip 32*(m%4).
    w_ps = [wp.tile([P, 512], F32, tag=f"w{i}", name=f"w_ps{i}") for i in range(2)]
    for i in range(2):
        nc.vector.memset(w_ps[i], 0.0)

    def w_slot(m):
        return w_ps[m // 4], 32 * (m % 4)

    def emit_chunk(qi, ci, Es, rs):
        t0, wid, eng = CHUNKS[ci]
        ps = pp.tile([P, 1536], F32, tag="ps", name=f"ps_s{qi}_{ci}")
        for h in range(wid // 512):
            c0 = t0 + h * 512
            for ei in range(2):
                nc.tensor.matmul(
                    ps[:, h * 512 : (h + 1) * 512],
                    qt_sb[:, ei, qi * P : (qi + 1) * P],
                    xt_slice(ei, c0, 512),
                    start=(ei == 0),
                    stop=(ei == 1),
                )
        if eng == "act":
            nc.scalar.activation(
                out=Es[:, t0 : t0 + wid],
                in_=ps[:, :wid],
                func=mybir.ActivationFunctionType.Exp,
                scale=1.0 / NORM,
                accum_out=rs[:, ci : ci + 1],
            )
        else:
            y = ypool.tile([P, 1024], I32, tag="y", name=f"y{qi}_{ci}")
            nc.vector.tensor_scalar(
                y[:, :wid],
                ps[:, :wid],
                SCH_A,
                SCH_B,
                mybir.AluOpType.mult,
                mybir.AluOpType.add,
            )
            nc.vector.tensor_scalar(
                Es[:, t0 : t0 + wid],
                y.bitcast(F32)[:, :wid],
                1.0,
                None,
                mybir.AluOpType.mult,
                mybir.AluOpType.add,
                accum_out=rs[:, ci : ci + 1],
            )

    def emit_recip(qi, rs):
        rsum = rsp.tile([P, 1], F32, tag="rsum", name=f"rsum{qi}")
        nc.vector.reduce_sum(out=rsum, in_=rs, axis=mybir.AxisListType.X)
        recf = rsp.tile([P, 1], F32, tag="recf", name=f"recf{qi}")
        nc.vector.reciprocal(out=recf, in_=rsum)
        recb = rsp.tile([P, 1], BF16, tag="recb", name=f"recb{qi}")
        nc.vector.tensor_copy(out=recb, in_=recf)
        return recb

    def emit_colsum(qi, Es, recb, ms=range(8)):
        for m in ms:
            wt, strip = w_slot(m)
            nc.tensor.matmul(
                wt[strip : strip + 1, :],
                recb,
                Es[:, m * 512 : (m + 1) * 512],
                start=(qi == 0),
                stop=(qi == QTILES - 1),
                tile_position=(0, strip),
            )

    # software-pipeline colsum: its recip dependency chain (accum reads ->
    # reduce -> reciprocal -> cast) takes a few us; lag a couple of q-tiles
    pending = {}
    for qi in range(QTILES):
        Es = epool.tile([P, S], BF16, tag="E", name=f"E{qi}")
        rs = rsp.tile([P, len(CHUNKS)], F32, tag="rs", name=f"rs{qi}")
        # order c0 (ACT), c2 (DVE), colsum, c1 (ACT): the two slots' next
        # writers arrive after their consumers' release, and c0 first keeps
        # q-tile 0 off the critical path of the late xt DMA chunks
        emit_chunk(qi, 0, Es, rs)
        emit_chunk(qi, 2, Es, rs)
        if qi - COLSUM_LAG in pending:
            emit_colsum(qi - COLSUM_LAG, *pending.pop(qi - COLSUM_LAG))
        if qi == QTILES - 1 and qi - 1 in pending:
            # drain the backlog so only q-tile 15's colsum trails the loop
            emit_colsum(qi - 1, *pending.pop(qi - 1))
        emit_chunk(qi, 1, Es, rs)
        if qi == 1:
            # fills the PE during the early pipeline-warmup stalls
            for eo in range(2):
                proj_unit(eo, 1)
        pending[qi] = (Es, emit_recip(qi, rs))
    for qi in sorted(pending):
        emit_colsum(qi, *pending.pop(qi))
    # keep the PE array's HAM clock-gate warm through the ~2.5us DVE
    # transpose chain so the tail matvec runs at full rate; outputs unread
    warm = pp.tile([P, 1536], F32, tag="ps", name="warm")
    for h in range(6):
        nc.tensor.matmul(
            warm[:, (h % 3) * 512 : (h % 3) * 512 + 512],
            qt_sb[:, 0, 0:P],
            xt_slice(0, h * 512, 512),
            start=True,
            stop=True,
        )

    # ---- tail: w strips -> SBUF, DVE 32x32 block-transpose to w^T columns,
    # matvec against xn (see _in_maps for the index mapping). Processed in
    # two halves (w_ps[0] -> matvec cols 0:16, w_ps[1] -> 16:32) so the
    # first half's matvec overlaps the second half's transpose.
    w_sb_f = const.tile([P, 1024], F32, name="w_sb_f")
    w_tr = const.tile([P, 1024], F32, name="w_tr")
    wt_sb = const.tile([P, 32], BF16, name="wt_sb")
    fin = None
    for half in range(2):
        h0 = half * 512
        if half == 1:
            # ACT copies the second half in parallel with DVE's first-half
            # transpose chain
            nc.scalar.copy(out=w_sb_f[:, h0 : h0 + 512], in_=w_ps[half])
        else:
            nc.vector.tensor_copy(out=w_sb_f[:, h0 : h0 + 512], in_=w_ps[half])
        if half == 0:
            # reuses w_ps[0]'s bank -- safe: it has just been copied out
            fin = wp.tile([P, 512], F32, tag="w0", name="fin")
            nc.vector.memset(fin[:, 0:E], 0.0)
        nc.vector.transpose(
            out=w_tr[:, h0 : h0 + 512], in_=w_sb_f[:, h0 : h0 + 512]
        )
        src = bass.AP(
            tensor=w_tr.tensor,
            offset=w_tr.offset + h0,
            ap=[w_tr.ap[0], [32, 16]],
        )
        nc.vector.tensor_copy(out=wt_sb[:, half * 16 : half * 16 + 16], in_=src)
        # final matvec u = w @ x: 4 concurrent column strips accumulate
        # partials at partitions {0,32,64,96}; combined on the host
        for c in range(16):
            col = half * 16 + c
            strip = 32 * (col % 4)
            nc.tensor.matmul(
                fin[strip : strip + 1, 0:E],
                wt_sb[:, col : col + 1],
                xn_sb[:, col, :],
                start=(col < 4),
                stop=(col >= 28),
                tile_position=(0, strip),
            )
    strips_sb = const.tile([P, E], F32, name="strips_sb")
    nc.vector.tensor_copy(out=strips_sb, in_=fin[:, 0:E])
    src_ap = bass.AP(
        tensor=strips_sb.tensor,
        offset=strips_sb.offset,
        ap=[[strips_sb.ap[0][0] * 32, 4], [1, E]],
    )
    nc.sync.dma_start(out=out_d[:, :], in_=src_ap)


_NC_CACHE = None


def _build_nc():
    global _NC_CACHE
    if _NC_CACHE is None:
        from contextlib import ExitStack

        nc = bacc.Bacc("TRN2", target_bir_lowering=False, debug=False)
        with tile.TileContext(nc) as tc, ExitStack() as ctx:
            _emit(ctx, tc)
        nc.compile()
        _NC_CACHE = nc
    return _NC_CACHE


def _in_maps(inputs):
    import ml_dtypes

    bf16 = ml_dtypes.bfloat16
    x = np.asarray(inputs["x"], dtype=np.float32)
    Wq = np.asarray(inputs["Wq"], dtype=np.float32)
    Wk = np.asarray(inputs["Wk"], dtype=np.float32)
    M = (Wq @ Wk.T).astype(np.float32)
    g = (np.asarray(inputs["bq"], dtype=np.float32) @ Wk.T).astype(np.float32)
    # p-major: wall[p, c, e] = M[128c + p, e]
    wall = np.ascontiguousarray(
        np.stack([M[:P, :], M[P:, :]], axis=1)
    ).astype(bf16)
    # bqc[p, a] = g[128a + p]
    bqc = np.ascontiguousarray(g.reshape(2, P).T)
    # xn tile cc, partition p holds x row t = 2048(cc//16) + 512(p//32)
    # + 32(cc%16) + (p%32), matching the DVE-block-transposed w layout
    p = np.arange(P)[:, None]
    cc = np.arange(S // P)[None, :]
    tidx = 2048 * (cc // 16) + 512 * (p // 32) + 32 * (cc % 16) + (p % 32)
    maps = []
    for c in range(N_CORES):
        b, h = divmod(c, 2)
        xr = np.roll(x[b], -h * HALF, axis=0)
        xt = np.ascontiguousarray(xr.T).astype(bf16)
        xn = np.ascontiguousarray(xr[tidx]).astype(bf16)
        maps.append({"xt": xt, "xn": xn, "wall": wall, "bqc": bqc})
    return maps


def _combine(results, inputs):
    Wv = np.asarray(inputs["Wv"], dtype=np.float32)
    bv = np.asarray(inputs["bv"], dtype=np.float32).reshape(E)
    parts = [r["out"].sum(axis=0) for r in results]
    out = np.stack(
        [((parts[2 * b] + parts[2 * b + 1]) / S) @ Wv + bv for b in range(B)]
    )[:, None, :]
    return out.astype(np.float32)


def kernel(**inputs):
    from concourse.bass_utils import run_bass_kernel_spmd

    nc = _build_nc()
    res = run_bass_kernel_spmd(nc, _in_maps(inputs), core_ids=list(range(N_CORES)))
    return _combine(res.results, inputs)
